# revision 46
# baseline (speedup 1.0000x reference)
"""Trainium2 Bass kernel: DynamicMoERoutingLayer (moe_routing).

Reference computes: routing projection -> cosine-sim vs 10 expert embeddings ->
softmax weights -> 10 expert 3x3 VALID convs -> weighted combine.

Key algebraic rewrite: conv is linear in its weights, so
    sum_n w[b,n] * conv(x_b, W_n)  ==  conv(x_b, sum_n w[b,n] * W_n)
We therefore combine the 10 expert kernels into ONE per-image kernel on device
(10x less conv compute), then run a single 3x3 conv per image.

Distribution: data-parallel over batch, 4 images per core (8 cores).

Conv-as-matmul with row-pair K-packing: each image's x lives in a [128, 4352]
bf16 tile X2 whose partitions 0-63 hold the 64 input channels (flat pixel
space y*64+x) and partitions 64-127 hold the SAME channels shifted one image
row (+64 px).  A matmul at column offset dx then contracts kernel rows 0 AND 1
in one K=128 pass; kernel row 2 needs a separate K=64 pass at offset 128+dx.
So a 3x3 conv costs 6 matmul slots per 512-px chunk instead of 9.
Two images (A, B) run concurrently on disjoint PE-array column halves
(tile_position (0,0) vs (0,64)), so the array stays fully busy.

The flat-pixel formulation computes 64x64 output positions per image; the 2
garbage columns/rows (VALID conv is 62x62) are trimmed on the host.

Precision: x is cast fp32->bf16 on ScalarE after plain fp32 HWDGE loads;
combined weights are accumulated in fp32 on VectorE then cast to bf16;
conv matmuls are bf16 with fp32 PSUM accumulation; routing math is fp32.

Hardware/toolchain constraints honored:
- A Matmult can carry only ONE semaphore wait (walrus ISA): Bacc's
  compile() passes legalize the rest, and PE-queue NOPs with dependency APs
  (the Tile-sanctioned mechanism) absorb cross-engine waits up front.
- HWDGE DMA descriptors only get the fast 16-engine spray for full-tile-width
  destinations with non-overlapping source rows; everything else runs ~10x
  slower, so all loads are shaped accordingly.
- fp32r matmuls are rejected at tile_position != (0,0), hence bf16.
"""

import functools
import os
import sys

import numpy as np

for _p in ("/opt/trn_rl_repo",):
    if os.path.isdir(_p) and _p not in sys.path:
        sys.path.insert(0, _p)

import concourse.bacc as bacc
import concourse.bass as bass
import concourse.mybir as mybir
import concourse.tile as tile
from concourse.bass_utils import run_bass_kernel_spmd

FP = mybir.dt.float32
BF = mybir.dt.bfloat16
AF = mybir.ActivationFunctionType
OP = mybir.AluOpType

N_CORES = 8
B = 32
B_LOC = B // N_CORES          # images per core
NPAIR = B_LOC // 2
CIN = 64
COUT = 64
PIX = 64 * 64                 # flat pixels computed per image (incl. garbage)
XCOLS = 4352                  # X2 columns (top needs 4226, bottom 4160)
XPAD = 256                    # tail pad elements of the flat x upload
BOT = 4160                    # bottom-half valid columns (>= 3584+2+512)
NEXP = 10
D = 128
R = 512
CHUNK = 512
NCHUNK = PIX // CHUNK         # 8
CWF = 384                     # combined weights: 192 pair-taps + 192 row-2
WAVE = 3                      # chunks per wave (PSUM banks: 6 conv + 2 routing)

# const-blob column layout
C_RPW = 0                     # [128, 4, 128]
C_RV = 512                    # [128, 4, 4]
C_RPB = 528                   # [128, 1]
C_ID = 529                    # [128, 128] identity
C_EMB = 657                   # [10, 128] on partitions 0..9
C_SELP = 785                  # [4, 2, 128] pair selector, partitions 0..3
C_SELI = 1041                 # [4, 4, 128] image selector, partitions 0..3
C_CBT = 1553                  # [128, 10]
CBLOB = 1568


def build_nc():
    # Bacc (not raw Bass): its compile() runs move_matmul_waits_to_ldweights +
    # generate_event_semaphores, which legalize multi-wait instructions for
    # the walrus ISA (each instruction carries at most one sync wait).
    nc = bacc.Bacc(None)

    x_d = nc.dram_tensor("x", [B_LOC * CIN * PIX + XPAD], FP,
                         kind="ExternalInput")
    cst_d = nc.dram_tensor("cst", [128, CBLOB], FP, kind="ExternalInput")
    base_d = nc.dram_tensor("base", [128, NEXP, CWF], FP,
                            kind="ExternalInput")
    out_d = nc.dram_tensor("out", [B_LOC, COUT, PIX], FP, kind="ExternalOutput")

    with tile.TileContext(nc) as tc:
        with (
            tc.tile_pool(name="consts", bufs=1) as consts,
            tc.tile_pool(name="xstage", bufs=2) as xstage,
            tc.tile_pool(name="x2p", bufs=4) as x2p,
            tc.tile_pool(name="cwp", bufs=4) as cwp,
            tc.tile_pool(name="outp", bufs=2) as outp,
            tc.tile_pool(name="scr", bufs=1) as scr,
            tc.tile_pool(name="rps", bufs=2, space="PSUM") as rps,
            tc.tile_pool(name="cps", bufs=2 * WAVE, space="PSUM") as cps,
        ):
            # activation-table warmup: pulls the lazy Sqrt/Exp table loads
            # (1.3 us each) off the routing critical path
            warm = scr.tile([1, 1], FP)
            nc.vector.memset(warm, 1.0)
            # Exp first: the table memory holds one set at a time, and the
            # routing path uses Sqrt (twice) before its single Exp, so warm
            # in the order that leaves Sqrt resident
            nc.scalar.activation(out=warm, in_=warm, func=AF.Exp)
            nc.scalar.activation(out=warm, in_=warm, func=AF.Sqrt)

            # ---- constant loads ------------------------------------------
            cst = consts.tile([128, CBLOB], FP)
            nc.sync.dma_start(out=cst, in_=cst_d[:])
            rpw_t = cst[:, C_RPW:C_RPW + 512].rearrange("p (k d) -> p k d", k=4)
            rv_t = cst[:, C_RV:C_RV + 16].rearrange("p (k b) -> p k b", k=4)
            rpb_t = cst[:, C_RPB:C_RPB + 1]
            ident = cst[:, C_ID:C_ID + 128]
            emb_t = cst[0:NEXP, C_EMB:C_EMB + 128]
            selp_t = cst[0:B_LOC, C_SELP:C_SELP + 256].rearrange(
                "b (p q) -> b p q", p=NPAIR)
            seli_t = cst[0:B_LOC, C_SELI:C_SELI + 512].rearrange(
                "b (i q) -> b i q", i=B_LOC)
            cbt_t = cst[:, C_CBT:C_CBT + NEXP]

            base_t = consts.tile([128, NEXP, CWF], FP)
            nc.sync.dma_start(out=base_t, in_=base_d[:])

            # x loads: one FULL-partition [128, PIX] DMA per image pair
            # ([64, N] tiles only get half the DMA ports => half bandwidth).
            # Even image's channels land on partitions 0-63, odd image's on
            # 64-127.
            xfull = x_d[:]
            x2 = []

            def build_pair(p):
                xm2 = xstage.tile([128, PIX], FP, name="xm2", tag="xm2")
                nc.sync.dma_start(out=xm2, in_=bass.AP(
                    tensor=xfull.tensor, offset=xfull.offset + p * 128 * PIX,
                    ap=[[PIX, 128], [1, PIX]]))
                xq2 = xstage.tile([128, XPAD], FP, name="xq2", tag="xq2")
                nc.sync.dma_start(out=xq2, in_=bass.AP(
                    tensor=xfull.tensor,
                    offset=xfull.offset + p * 128 * PIX + PIX,
                    ap=[[PIX, 128], [1, XPAD]]))

                # even image: cast straight into its X2 top (lanes 0-63)
                # (monolithic casts measured faster than chunked: per-op
                # overhead beats the scheduling-granularity benefit)
                x2a = x2p.tile([128, XCOLS], BF, name="x2a", tag="x2")
                nc.scalar.activation(out=x2a[0:64, 0:PIX], in_=xm2[0:64, :],
                                     func=AF.Copy)
                nc.scalar.activation(out=x2a[0:64, PIX:XCOLS],
                                     in_=xq2[0:64, :], func=AF.Copy)
                nc.sync.dma_start(out=x2a[64:128, 0:BOT],
                                  in_=x2a[0:64, 64:64 + BOT])
                x2.append(x2a)

                # odd image: cast in-place at lanes 64-127 into a tmp, then
                # two parallel SBUF shifts build top and (shifted) bottom
                tmp = x2p.tile([128, XCOLS], BF, name="xtmp", tag="xtmp")
                nc.scalar.activation(out=tmp[64:128, 0:PIX],
                                     in_=xm2[64:128, :], func=AF.Copy)
                nc.scalar.activation(out=tmp[64:128, PIX:XCOLS],
                                     in_=xq2[64:128, :], func=AF.Copy)
                x2b = x2p.tile([128, XCOLS], BF, name="x2b", tag="x2")
                nc.sync.dma_start(out=x2b[0:64, 0:XCOLS],
                                  in_=tmp[64:128, 0:XCOLS])
                nc.sync.dma_start(out=x2b[64:128, 0:BOT],
                                  in_=tmp[64:128, 64:64 + BOT])
                x2.append(x2b)
                return xm2

            xm01 = build_pair(0)
            gate = mybir.InstNoOp(
                name=nc.get_next_instruction_name(), text_hint="xgate",
                ins=[nc.sync.lower_ap(xm01[:, 0:1])])
            nc.sync.add_instruction(gate)
            build_pair(1)

            # ---- routing: r = rv @ rp_w.T + rp_b  (D on partitions) -------
            r_ps = rps.tile([128, B_LOC], FP, tag="r")
            for k0 in range(R // 128):
                nc.tensor.matmul(r_ps, lhsT=rpw_t[:, k0, :], rhs=rv_t[:, k0, :],
                                 start=(k0 == 0), stop=(k0 == R // 128 - 1))
            rT = scr.tile([128, B_LOC], FP)
            nc.vector.tensor_scalar(out=rT, in0=r_ps, scalar1=rpb_t,
                                    scalar2=None, op0=OP.add)

            # ||r_b||: transpose r to [b, d] then square+row-sum
            r4_ps = rps.tile([B_LOC, 128], FP, tag="r")
            nc.tensor.transpose(r4_ps, rT, ident)
            r4 = scr.tile([B_LOC, 128], FP)
            nc.vector.tensor_copy(r4, r4_ps)
            rsq = scr.tile([B_LOC, 128], FP)
            rn2 = scr.tile([B_LOC, 1], FP)
            nc.vector.scalar_tensor_tensor(out=rsq, in0=r4, scalar=1.0,
                                           in1=r4, op0=OP.mult, op1=OP.mult,
                                           accum_out=rn2)
            rnorm = scr.tile([B_LOC, 1], FP)
            nc.scalar.activation(out=rnorm, in_=rn2, func=AF.Sqrt)
            rinv = scr.tile([B_LOC, 1], FP)
            nc.vector.reciprocal(rinv, rnorm)

            # normalized embeddings, then transpose to [d, n]
            esq = scr.tile([NEXP, D], FP)
            en2 = scr.tile([NEXP, 1], FP)
            nc.vector.scalar_tensor_tensor(out=esq, in0=emb_t, scalar=1.0,
                                           in1=emb_t, op0=OP.mult, op1=OP.mult,
                                           accum_out=en2)
            enorm = scr.tile([NEXP, 1], FP)
            nc.scalar.activation(out=enorm, in_=en2, func=AF.Sqrt)
            einv = scr.tile([NEXP, 1], FP)
            nc.vector.reciprocal(einv, enorm)
            ehat = scr.tile([NEXP, D], FP)
            nc.vector.tensor_scalar(out=ehat, in0=emb_t, scalar1=einv,
                                    scalar2=None, op0=OP.mult)
            ehatT_ps = rps.tile([D, NEXP], FP, tag="r")
            nc.tensor.transpose(ehatT_ps, ehat, ident[:NEXP, :NEXP])
            ehatT = scr.tile([D, NEXP], FP)
            nc.vector.tensor_copy(ehatT, ehatT_ps)

            # cosine sim [b, n] and softmax over n.  Fusions: sims are
            # bounded in [-1,1] so no max-subtraction is needed; the 1/||r||
            # scale rides the Exp's per-partition scale operand; and the
            # 1/sum(exp) normalization moves OFF the critical path — the
            # weight MACs use unnormalized exp weights and the conv epilogue
            # rescales by a pair-stacked 1/sum (its `scale` operand).
            dot_ps = rps.tile([B_LOC, NEXP], FP, tag="r")
            nc.tensor.matmul(dot_ps, lhsT=rT, rhs=ehatT, start=True, stop=True)
            ex = scr.tile([B_LOC, NEXP], FP)
            sume = scr.tile([B_LOC, 1], FP)
            nc.scalar.activation(out=ex, in_=dot_ps, func=AF.Exp,
                                 scale=rinv[:, 0:1], accum_out=sume)
            sinv = scr.tile([B_LOC, 1], FP)
            nc.vector.reciprocal(sinv, sume)
            wts = ex

            # routing weights broadcast to all 128 partitions via selector
            # matmuls: per-image (for the weight MACs) and pair-stacked
            # (for the combined bias)
            w128_ps = rps.tile([128, B_LOC + NPAIR, NEXP], FP, tag="r")
            for i in range(B_LOC):
                nc.tensor.matmul(w128_ps[:, i, :], lhsT=seli_t[:, i, :],
                                 rhs=wts, start=True, stop=True)
            for p in range(NPAIR):
                nc.tensor.matmul(w128_ps[:, B_LOC + p, :],
                                 lhsT=selp_t[:, p, :], rhs=wts,
                                 start=True, stop=True)
            w128 = consts.tile([128, B_LOC + NPAIR, NEXP], FP)
            nc.vector.tensor_copy(w128, w128_ps)

            # pair-stacked 1/sum(exp) for the epilogue scale operand
            scl_ps = rps.tile([128, NPAIR], FP, tag="r")
            for p in range(NPAIR):
                nc.tensor.matmul(scl_ps[:, p:p + 1], lhsT=selp_t[:, p, :],
                                 rhs=sinv, start=True, stop=True)
            scl2 = consts.tile([128, NPAIR], FP)
            nc.vector.tensor_copy(scl2, scl_ps)

            # combined conv bias, pair-stacked [128, pair]:
            # bias2[part, p] = sum_n wts[2p + part//64, n]*conv_b[n, part%64]
            bias2 = consts.tile([128, NPAIR], FP)
            bscrap = scr.tile([128, NEXP], FP)
            for p in range(NPAIR):
                nc.vector.scalar_tensor_tensor(
                    out=bscrap, in0=w128[:, B_LOC + p, :], scalar=1.0,
                    in1=cbt_t, op0=OP.mult, op1=OP.mult,
                    accum_out=bias2[:, p:p + 1])
            # bias was combined from unnormalized weights: rescale by 1/sum
            nc.vector.tensor_mul(bias2, bias2, scl2)

            # per-image combined conv weights (fp32 MACs) + bf16 cast.
            # Images 2/3's chains are gated behind pair 0's finished weights
            # (a DVE-queue nop with dep APs): otherwise the work-conserving
            # scheduler interleaves all four chains and pair 0's conv start
            # slips by ~10 us.
            cwb = []
            for i in range(B_LOC):
                if i == 2:
                    mgate = mybir.InstNoOp(
                        name=nc.get_next_instruction_name(), text_hint="mgate",
                        ins=[nc.vector.lower_ap(cwb[0][:, 0:1]),
                             nc.vector.lower_ap(cwb[1][:, 0:1])])
                    nc.vector.add_instruction(mgate)
                cw = cwp.tile([128, CWF], FP, name="cw", tag=f"cw{i % 2}")
                nc.vector.tensor_scalar(out=cw, in0=base_t[:, 0, :],
                                        scalar1=w128[:, i, 0:1], scalar2=None,
                                        op0=OP.mult)
                for n in range(1, NEXP):
                    nc.vector.scalar_tensor_tensor(
                        out=cw, in0=base_t[:, n, :], scalar=w128[:, i, n:n + 1],
                        in1=cw, op0=OP.mult, op1=OP.add)
                cwbi = cwp.tile([128, CWF], BF, name="cwb", tag="cwb")
                nc.vector.tensor_copy(cwbi, cw)
                cwb.append(cwbi)

            # ---- per-pair conv ------------------------------------------
            for p in range(NPAIR):
                iA, iB = 2 * p, 2 * p + 1
                outt = outp.tile([128, PIX], FP)
                for w0 in range(0, NCHUNK, WAVE):
                    chunks = list(range(w0, min(w0 + WAVE, NCHUNK)))
                    pst = {c: cps.tile([128, CHUNK], FP, name="pst")
                           for c in chunks}
                    # PE-queue NOP absorbs all cross-engine waits (psum bank
                    # release, X2 casts+shift-DMA, cwb) so each Matmult needs
                    # at most its single legal wait
                    dep = mybir.InstNoOp(
                        name=nc.get_next_instruction_name(), text_hint="dep",
                        ins=[nc.tensor.lower_ap(x2[iA][:, 0:1]),
                             nc.tensor.lower_ap(x2[iA][0:64, PIX:PIX + 1]),
                             nc.tensor.lower_ap(x2[iB][:, 0:1]),
                             nc.tensor.lower_ap(x2[iB][0:64, PIX:PIX + 1]),
                             nc.tensor.lower_ap(cwb[iA][:, 0:1]),
                             nc.tensor.lower_ap(cwb[iB][:, 0:1])],
                        outs=[nc.tensor.lower_ap(pst[c]) for c in chunks],
                    )
                    nc.tensor.add_instruction(dep)
                    # phase 1: kernel rows 0+1 in one K=128 pass per dx
                    for dx in range(3):
                        for c in chunks:
                            lo = c * CHUNK + dx
                            for half, img in ((0, iA), (1, iB)):
                                sl = slice(64 * half, 64 * half + 64)
                                nc.tensor.matmul(
                                    pst[c][sl, :],
                                    lhsT=cwb[img][0:128, dx * 64:dx * 64 + 64],
                                    rhs=x2[img][0:128, lo:lo + CHUNK],
                                    start=(dx == 0), stop=False,
                                    skip_group_check=True)
                    # phase 2: kernel row 2, K=64 from the top half only
                    # (weights always on array rows 0-63: tile positions
                    # beyond (0,0)/(0,64) proved unreliable on silicon)
                    for dx in range(3):
                        for c in chunks:
                            lo = c * CHUNK + 128 + dx
                            for half, img in ((0, iA), (1, iB)):
                                sl = slice(64 * half, 64 * half + 64)
                                nc.tensor.matmul(
                                    pst[c][sl, :],
                                    lhsT=cwb[img][0:64,
                                                  192 + dx * 64:256 + dx * 64],
                                    rhs=x2[img][0:64, lo:lo + CHUNK],
                                    start=False, stop=(dx == 2),
                                    skip_group_check=True)
                    for c in chunks:
                        nc.scalar.activation(
                            out=outt[:, c * CHUNK:(c + 1) * CHUNK],
                            in_=pst[c], func=AF.Identity,
                            bias=bias2[:, p:p + 1],
                            scale=scl2[:, p:p + 1])
                    dst = out_d[2 * p:2 * p + 2].flatten_outer_dims()
                    lo, hi = w0 * CHUNK, (chunks[-1] + 1) * CHUNK
                    nc.sync.dma_start(out=dst[:, lo:hi], in_=outt[:, lo:hi])

    nc.compile()
    return nc


@functools.lru_cache(maxsize=1)
def _nc_cached():
    return build_nc()


def _prep_in_maps(inputs):
    x = np.asarray(inputs["x"], dtype=np.float32).reshape(B, CIN, PIX)
    rv = np.asarray(inputs["routing_vector"], dtype=np.float32)
    conv_w = np.asarray(inputs["conv_w"], dtype=np.float32)
    conv_b = np.asarray(inputs["conv_b"], dtype=np.float32)
    emb = np.asarray(inputs["emb"], dtype=np.float32)
    rp_w = np.asarray(inputs["rp_w"], dtype=np.float32)
    rp_b = np.asarray(inputs["rp_b"], dtype=np.float32)

    # base layout for the stacked-tap lhsT (see module docstring):
    #   cols 0:192  : [p = cin + 64*dy(0/1), n, dx*64 + cout]
    #   cols 192:288: [p = cin (0..63),      n, dx*64 + cout]  (kernel row 2)
    base = np.zeros((128, NEXP, CWF), np.float32)
    b01 = conv_w[:, :, :, 0:2, :].transpose(3, 2, 0, 4, 1)  # dy,c,n,dx,m
    base[:, :, 0:192] = b01.reshape(128, NEXP, 192)
    b2 = conv_w[:, :, :, 2, :].transpose(2, 0, 3, 1)        # c,n,dx,m
    base[0:64, :, 192:384] = b2.reshape(64, NEXP, 192)

    blob = np.zeros((128, CBLOB), np.float32)
    blob[:, C_RPW:C_RPW + 512] = (
        rp_w.T.reshape(4, 128, D).transpose(1, 0, 2).reshape(128, 512))
    blob[:, C_RPB] = rp_b
    blob[:, C_ID:C_ID + 128] = np.eye(128, dtype=np.float32)
    blob[0:NEXP, C_EMB:C_EMB + 128] = emb
    selp = np.zeros((B_LOC, NPAIR, 128), np.float32)
    for p in range(NPAIR):
        selp[2 * p, p, 0:64] = 1.0
        selp[2 * p + 1, p, 64:128] = 1.0
    blob[0:B_LOC, C_SELP:C_SELP + 256] = selp.reshape(B_LOC, 256)
    seli = np.zeros((B_LOC, B_LOC, 128), np.float32)
    for i in range(B_LOC):
        seli[i, i, :] = 1.0
    blob[0:B_LOC, C_SELI:C_SELI + 512] = seli.reshape(B_LOC, 512)
    blob[:, C_CBT:C_CBT + NEXP] = np.tile(conv_b.T, (2, 1))

    in_maps = []
    for c in range(N_CORES):
        sl = slice(B_LOC * c, B_LOC * (c + 1))
        cblob = blob.copy()
        cblob[:, C_RV:C_RV + 16] = (
            rv[sl].T.reshape(4, 128, B_LOC).transpose(1, 0, 2).reshape(128, 16))
        in_maps.append({
            "x": np.concatenate([x[sl].reshape(-1),
                                 np.zeros(XPAD, np.float32)]),
            "cst": cblob,
            "base": base,
        })
    return in_maps


def run(inputs, trace=False, **kw):
    """Returns (full_output, BassKernelResults)."""
    nc = _nc_cached()
    in_maps = _prep_in_maps(inputs)
    res = run_bass_kernel_spmd(nc, in_maps, core_ids=list(range(N_CORES)),
                               trace=trace, **kw)
    outs = [r["out"].reshape(B_LOC, COUT, 64, 64)[:, :, :62, :62]
            for r in res.results]
    return np.concatenate(outs, axis=0), res


def kernel(**inputs):
    out, _ = run(inputs, trace=False)
    return out


# revision 47
# speedup vs baseline: 1.0617x; 1.0617x over previous
"""Trainium2 Bass kernel: DynamicMoERoutingLayer (moe_routing).

Reference computes: routing projection -> cosine-sim vs 10 expert embeddings ->
softmax weights -> 10 expert 3x3 VALID convs -> weighted combine.

Key algebraic rewrite: conv is linear in its weights, so
    sum_n w[b,n] * conv(x_b, W_n)  ==  conv(x_b, sum_n w[b,n] * W_n)
We therefore combine the 10 expert kernels into ONE per-image kernel on device
(10x less conv compute), then run a single 3x3 conv per image.

Distribution: data-parallel over batch, 4 images per core (8 cores).

Conv-as-matmul with row-pair K-packing: each image's x lives in a [128, 4352]
bf16 tile X2 whose partitions 0-63 hold the 64 input channels (flat pixel
space y*64+x) and partitions 64-127 hold the SAME channels shifted one image
row (+64 px).  A matmul at column offset dx then contracts kernel rows 0 AND 1
in one K=128 pass; kernel row 2 needs a separate K=64 pass at offset 128+dx.
So a 3x3 conv costs 6 matmul slots per 512-px chunk instead of 9.
Two images (A, B) run concurrently on disjoint PE-array column halves
(tile_position (0,0) vs (0,64)), so the array stays fully busy.

The flat-pixel formulation computes 64x64 output positions per image; the 2
garbage columns/rows (VALID conv is 62x62) are trimmed on the host.

Precision: x is cast fp32->bf16 on ScalarE after plain fp32 HWDGE loads;
combined weights are accumulated in fp32 on VectorE then cast to bf16;
conv matmuls are bf16 with fp32 PSUM accumulation; routing math is fp32.

Hardware/toolchain constraints honored:
- A Matmult can carry only ONE semaphore wait (walrus ISA): Bacc's
  compile() passes legalize the rest, and PE-queue NOPs with dependency APs
  (the Tile-sanctioned mechanism) absorb cross-engine waits up front.
- HWDGE DMA descriptors only get the fast 16-engine spray for full-tile-width
  destinations with non-overlapping source rows; everything else runs ~10x
  slower, so all loads are shaped accordingly.
- fp32r matmuls are rejected at tile_position != (0,0), hence bf16.
"""

import functools
import os
import sys

import numpy as np

for _p in ("/opt/trn_rl_repo",):
    if os.path.isdir(_p) and _p not in sys.path:
        sys.path.insert(0, _p)

import concourse.bacc as bacc
import concourse.bass as bass
import concourse.mybir as mybir
import concourse.tile as tile
from concourse.bass_utils import run_bass_kernel_spmd

FP = mybir.dt.float32
BF = mybir.dt.bfloat16
AF = mybir.ActivationFunctionType
OP = mybir.AluOpType

N_CORES = 8
B = 32
B_LOC = B // N_CORES          # images per core
NPAIR = B_LOC // 2
CIN = 64
COUT = 64
PIX = 64 * 64                 # flat pixels computed per image (incl. garbage)
XCOLS = 4352                  # X2 columns (top needs 4226, bottom 4160)
XPAD = 256                    # tail pad elements of the flat x upload
BOT = 4160                    # bottom-half valid columns (>= 3584+2+512)
NEXP = 10
D = 128
R = 512
CHUNK = 512
NCHUNK = PIX // CHUNK         # 8
CWF = 384                     # combined weights: 192 pair-taps + 192 row-2
WAVE = 3                      # chunks per wave (PSUM banks: 6 conv + 2 routing)

# const-blob column layout
C_RPW = 0                     # [128, 4, 128]
C_RV = 512                    # [128, 4, 4]
C_RPB = 528                   # [128, 1]
C_ID = 529                    # [128, 128] identity
C_EMB = 657                   # [10, 128] on partitions 0..9
C_SELP = 785                  # [4, 2, 128] pair selector, partitions 0..3
C_SELI = 1041                 # [4, 4, 128] image selector, partitions 0..3
C_CBT = 1553                  # [128, 10]
CBLOB = 1568


def build_nc():
    # Bacc (not raw Bass): its compile() runs move_matmul_waits_to_ldweights +
    # generate_event_semaphores, which legalize multi-wait instructions for
    # the walrus ISA (each instruction carries at most one sync wait).
    nc = bacc.Bacc(None)

    x_d = nc.dram_tensor("x", [B_LOC * CIN * PIX + XPAD], FP,
                         kind="ExternalInput")
    cst_d = nc.dram_tensor("cst", [128, CBLOB], FP, kind="ExternalInput")
    base_d = nc.dram_tensor("base", [128, NEXP, CWF], FP,
                            kind="ExternalInput")
    out_d = nc.dram_tensor("out", [B_LOC, COUT, PIX], FP, kind="ExternalOutput")

    with tile.TileContext(nc) as tc:
        with (
            tc.tile_pool(name="consts", bufs=1) as consts,
            tc.tile_pool(name="xstage", bufs=2) as xstage,
            tc.tile_pool(name="x2p", bufs=4) as x2p,
            tc.tile_pool(name="cwp", bufs=4) as cwp,
            tc.tile_pool(name="outp", bufs=2) as outp,
            tc.tile_pool(name="scr", bufs=1) as scr,
            tc.tile_pool(name="rps", bufs=2, space="PSUM") as rps,
            tc.tile_pool(name="cps", bufs=2 * WAVE, space="PSUM") as cps,
        ):
            # activation-table warmup: pulls the lazy Sqrt/Exp table loads
            # (1.3 us each) off the routing critical path
            warm = scr.tile([1, 1], FP)
            nc.vector.memset(warm, 1.0)
            # Exp first: the table memory holds one set at a time, and the
            # routing path uses Sqrt (twice) before its single Exp, so warm
            # in the order that leaves Sqrt resident
            nc.scalar.activation(out=warm, in_=warm, func=AF.Exp)
            nc.scalar.activation(out=warm, in_=warm, func=AF.Sqrt)

            # ---- constant loads ------------------------------------------
            cst = consts.tile([128, CBLOB], FP)
            nc.sync.dma_start(out=cst, in_=cst_d[:])
            rpw_t = cst[:, C_RPW:C_RPW + 512].rearrange("p (k d) -> p k d", k=4)
            rv_t = cst[:, C_RV:C_RV + 16].rearrange("p (k b) -> p k b", k=4)
            rpb_t = cst[:, C_RPB:C_RPB + 1]
            ident = cst[:, C_ID:C_ID + 128]
            emb_t = cst[0:NEXP, C_EMB:C_EMB + 128]
            selp_t = cst[0:B_LOC, C_SELP:C_SELP + 256].rearrange(
                "b (p q) -> b p q", p=NPAIR)
            seli_t = cst[0:B_LOC, C_SELI:C_SELI + 512].rearrange(
                "b (i q) -> b i q", i=B_LOC)
            cbt_t = cst[:, C_CBT:C_CBT + NEXP]

            base_t = consts.tile([128, NEXP, CWF], FP)
            nc.sync.dma_start(out=base_t, in_=base_d[:])

            # x loads: one FULL-partition [128, PIX] DMA per image pair
            # ([64, N] tiles only get half the DMA ports => half bandwidth).
            # Even image's channels land on partitions 0-63, odd image's on
            # 64-127.
            xfull = x_d[:]
            x2 = []

            def build_pair(p):
                xm2 = xstage.tile([128, PIX], FP, name="xm2", tag="xm2")
                nc.sync.dma_start(out=xm2, in_=bass.AP(
                    tensor=xfull.tensor, offset=xfull.offset + p * 128 * PIX,
                    ap=[[PIX, 128], [1, PIX]]))
                xq2 = xstage.tile([128, XPAD], FP, name="xq2", tag="xq2")
                nc.sync.dma_start(out=xq2, in_=bass.AP(
                    tensor=xfull.tensor,
                    offset=xfull.offset + p * 128 * PIX + PIX,
                    ap=[[PIX, 128], [1, XPAD]]))

                # even image: cast straight into its X2 top (lanes 0-63)
                # (monolithic casts measured faster than chunked: per-op
                # overhead beats the scheduling-granularity benefit)
                x2a = x2p.tile([128, XCOLS], BF, name="x2a", tag="x2")
                nc.scalar.activation(out=x2a[0:64, 0:PIX], in_=xm2[0:64, :],
                                     func=AF.Copy)
                nc.scalar.activation(out=x2a[0:64, PIX:XCOLS],
                                     in_=xq2[0:64, :], func=AF.Copy)
                nc.sync.dma_start(out=x2a[64:128, 0:BOT],
                                  in_=x2a[0:64, 64:64 + BOT])
                x2.append(x2a)

                # odd image: cast in-place at lanes 64-127 into a tmp, then
                # two parallel SBUF shifts build top and (shifted) bottom
                tmp = x2p.tile([128, XCOLS], BF, name="xtmp", tag="xtmp")
                nc.scalar.activation(out=tmp[64:128, 0:PIX],
                                     in_=xm2[64:128, :], func=AF.Copy)
                nc.scalar.activation(out=tmp[64:128, PIX:XCOLS],
                                     in_=xq2[64:128, :], func=AF.Copy)
                x2b = x2p.tile([128, XCOLS], BF, name="x2b", tag="x2")
                nc.sync.dma_start(out=x2b[0:64, 0:XCOLS],
                                  in_=tmp[64:128, 0:XCOLS])
                nc.sync.dma_start(out=x2b[64:128, 0:BOT],
                                  in_=tmp[64:128, 64:64 + BOT])
                x2.append(x2b)
                return xm2

            xm01 = build_pair(0)
            gate = mybir.InstNoOp(
                name=nc.get_next_instruction_name(), text_hint="xgate",
                ins=[nc.sync.lower_ap(xm01[:, 0:1])])
            nc.sync.add_instruction(gate)
            build_pair(1)

            # ---- routing: r = rv @ rp_w.T + rp_b  (D on partitions) -------
            r_ps = rps.tile([128, B_LOC], FP, tag="r")
            for k0 in range(R // 128):
                nc.tensor.matmul(r_ps, lhsT=rpw_t[:, k0, :], rhs=rv_t[:, k0, :],
                                 start=(k0 == 0), stop=(k0 == R // 128 - 1))
            rT = scr.tile([128, B_LOC], FP)
            nc.vector.tensor_scalar(out=rT, in0=r_ps, scalar1=rpb_t,
                                    scalar2=None, op0=OP.add)

            # ||r_b||: transpose r to [b, d] then square+row-sum
            r4_ps = rps.tile([B_LOC, 128], FP, tag="r")
            nc.tensor.transpose(r4_ps, rT, ident)
            r4 = scr.tile([B_LOC, 128], FP)
            nc.vector.tensor_copy(r4, r4_ps)
            rsq = scr.tile([B_LOC, 128], FP)
            rn2 = scr.tile([B_LOC, 1], FP)
            nc.vector.scalar_tensor_tensor(out=rsq, in0=r4, scalar=1.0,
                                           in1=r4, op0=OP.mult, op1=OP.mult,
                                           accum_out=rn2)
            rnorm = scr.tile([B_LOC, 1], FP)
            nc.scalar.activation(out=rnorm, in_=rn2, func=AF.Sqrt)
            rinv = scr.tile([B_LOC, 1], FP)
            nc.vector.reciprocal(rinv, rnorm)

            # normalized embeddings, then transpose to [d, n]
            esq = scr.tile([NEXP, D], FP)
            en2 = scr.tile([NEXP, 1], FP)
            nc.vector.scalar_tensor_tensor(out=esq, in0=emb_t, scalar=1.0,
                                           in1=emb_t, op0=OP.mult, op1=OP.mult,
                                           accum_out=en2)
            enorm = scr.tile([NEXP, 1], FP)
            nc.scalar.activation(out=enorm, in_=en2, func=AF.Sqrt)
            einv = scr.tile([NEXP, 1], FP)
            nc.vector.reciprocal(einv, enorm)
            ehat = scr.tile([NEXP, D], FP)
            nc.vector.tensor_scalar(out=ehat, in0=emb_t, scalar1=einv,
                                    scalar2=None, op0=OP.mult)
            ehatT_ps = rps.tile([D, NEXP], FP, tag="r")
            nc.tensor.transpose(ehatT_ps, ehat, ident[:NEXP, :NEXP])
            ehatT = scr.tile([D, NEXP], FP)
            nc.vector.tensor_copy(ehatT, ehatT_ps)

            # cosine sim [b, n] and softmax over n
            dot_ps = rps.tile([B_LOC, NEXP], FP, tag="r")
            nc.tensor.matmul(dot_ps, lhsT=rT, rhs=ehatT, start=True, stop=True)
            sim = scr.tile([B_LOC, NEXP], FP)
            nc.vector.tensor_scalar(out=sim, in0=dot_ps, scalar1=rinv,
                                    scalar2=None, op0=OP.mult)
            nmax = scr.tile([B_LOC, 1], FP)
            nc.vector.tensor_reduce(out=nmax, in_=sim,
                                    axis=mybir.AxisListType.X, op=OP.max,
                                    negate=True)
            ex = scr.tile([B_LOC, NEXP], FP)
            sume = scr.tile([B_LOC, 1], FP)
            nc.scalar.activation(out=ex, in_=sim, func=AF.Exp,
                                 bias=nmax[:, 0:1], scale=1.0, accum_out=sume)
            sinv = scr.tile([B_LOC, 1], FP)
            nc.vector.reciprocal(sinv, sume)
            wts = scr.tile([B_LOC, NEXP], FP)
            nc.vector.tensor_scalar(out=wts, in0=ex, scalar1=sinv,
                                    scalar2=None, op0=OP.mult)

            # routing weights broadcast to all 128 partitions via selector
            # matmuls: per-image (for the weight MACs) and pair-stacked
            # (for the combined bias)
            w128_ps = rps.tile([128, B_LOC + NPAIR, NEXP], FP, tag="r")
            for i in range(B_LOC):
                nc.tensor.matmul(w128_ps[:, i, :], lhsT=seli_t[:, i, :],
                                 rhs=wts, start=True, stop=True)
            for p in range(NPAIR):
                nc.tensor.matmul(w128_ps[:, B_LOC + p, :],
                                 lhsT=selp_t[:, p, :], rhs=wts,
                                 start=True, stop=True)
            w128 = consts.tile([128, B_LOC + NPAIR, NEXP], FP)
            nc.vector.tensor_copy(w128, w128_ps)

            # combined conv bias, pair-stacked [128, pair]:
            # bias2[part, p] = sum_n wts[2p + part//64, n]*conv_b[n, part%64]
            bias2 = consts.tile([128, NPAIR], FP)
            bscrap = scr.tile([128, NEXP], FP)
            for p in range(NPAIR):
                nc.vector.scalar_tensor_tensor(
                    out=bscrap, in0=w128[:, B_LOC + p, :], scalar=1.0,
                    in1=cbt_t, op0=OP.mult, op1=OP.mult,
                    accum_out=bias2[:, p:p + 1])

            # per-image combined conv weights (fp32 MACs) + bf16 cast.
            # Images 2/3's chains are gated behind pair 0's finished weights
            # (a DVE-queue nop with dep APs): otherwise the work-conserving
            # scheduler interleaves all four chains and pair 0's conv start
            # slips by ~10 us.
            cwb = []
            for i in range(B_LOC):
                if i == 2:
                    mgate = mybir.InstNoOp(
                        name=nc.get_next_instruction_name(), text_hint="mgate",
                        ins=[nc.vector.lower_ap(cwb[0][:, 0:1]),
                             nc.vector.lower_ap(cwb[1][:, 0:1])])
                    nc.vector.add_instruction(mgate)
                cw = cwp.tile([128, CWF], FP, name="cw", tag=f"cw{i % 2}")
                nc.vector.tensor_scalar(out=cw, in0=base_t[:, 0, :],
                                        scalar1=w128[:, i, 0:1], scalar2=None,
                                        op0=OP.mult)
                for n in range(1, NEXP):
                    nc.vector.scalar_tensor_tensor(
                        out=cw, in0=base_t[:, n, :], scalar=w128[:, i, n:n + 1],
                        in1=cw, op0=OP.mult, op1=OP.add)
                cwbi = cwp.tile([128, CWF], BF, name="cwb", tag="cwb")
                nc.vector.tensor_copy(cwbi, cw)
                cwb.append(cwbi)

            # ---- per-pair conv ------------------------------------------
            for p in range(NPAIR):
                iA, iB = 2 * p, 2 * p + 1
                outt = outp.tile([128, PIX], FP)
                for w0 in range(0, NCHUNK, WAVE):
                    chunks = list(range(w0, min(w0 + WAVE, NCHUNK)))
                    pst = {c: cps.tile([128, CHUNK], FP, name="pst")
                           for c in chunks}
                    # PE-queue NOP absorbs all cross-engine waits (psum bank
                    # release, X2 casts+shift-DMA, cwb) so each Matmult needs
                    # at most its single legal wait
                    dep = mybir.InstNoOp(
                        name=nc.get_next_instruction_name(), text_hint="dep",
                        ins=[nc.tensor.lower_ap(x2[iA][:, 0:1]),
                             nc.tensor.lower_ap(x2[iA][0:64, PIX:PIX + 1]),
                             nc.tensor.lower_ap(x2[iB][:, 0:1]),
                             nc.tensor.lower_ap(x2[iB][0:64, PIX:PIX + 1]),
                             nc.tensor.lower_ap(cwb[iA][:, 0:1]),
                             nc.tensor.lower_ap(cwb[iB][:, 0:1])],
                        outs=[nc.tensor.lower_ap(pst[c]) for c in chunks],
                    )
                    nc.tensor.add_instruction(dep)
                    # phase 1: kernel rows 0+1 in one K=128 pass per dx
                    for dx in range(3):
                        for c in chunks:
                            lo = c * CHUNK + dx
                            for half, img in ((0, iA), (1, iB)):
                                sl = slice(64 * half, 64 * half + 64)
                                nc.tensor.matmul(
                                    pst[c][sl, :],
                                    lhsT=cwb[img][0:128, dx * 64:dx * 64 + 64],
                                    rhs=x2[img][0:128, lo:lo + CHUNK],
                                    start=(dx == 0), stop=False,
                                    skip_group_check=True)
                    # phase 2: kernel row 2, K=64 from the top half only
                    # (weights always on array rows 0-63: tile positions
                    # beyond (0,0)/(0,64) proved unreliable on silicon)
                    for dx in range(3):
                        for c in chunks:
                            lo = c * CHUNK + 128 + dx
                            for half, img in ((0, iA), (1, iB)):
                                sl = slice(64 * half, 64 * half + 64)
                                nc.tensor.matmul(
                                    pst[c][sl, :],
                                    lhsT=cwb[img][0:64,
                                                  192 + dx * 64:256 + dx * 64],
                                    rhs=x2[img][0:64, lo:lo + CHUNK],
                                    start=False, stop=(dx == 2),
                                    skip_group_check=True)
                    for c in chunks:
                        nc.scalar.activation(
                            out=outt[:, c * CHUNK:(c + 1) * CHUNK],
                            in_=pst[c], func=AF.Identity,
                            bias=bias2[:, p:p + 1], scale=1.0)
                    dst = out_d[2 * p:2 * p + 2].flatten_outer_dims()
                    lo, hi = w0 * CHUNK, (chunks[-1] + 1) * CHUNK
                    nc.sync.dma_start(out=dst[:, lo:hi], in_=outt[:, lo:hi])

    nc.compile()
    return nc


@functools.lru_cache(maxsize=1)
def _nc_cached():
    return build_nc()


def _prep_in_maps(inputs):
    x = np.asarray(inputs["x"], dtype=np.float32).reshape(B, CIN, PIX)
    rv = np.asarray(inputs["routing_vector"], dtype=np.float32)
    conv_w = np.asarray(inputs["conv_w"], dtype=np.float32)
    conv_b = np.asarray(inputs["conv_b"], dtype=np.float32)
    emb = np.asarray(inputs["emb"], dtype=np.float32)
    rp_w = np.asarray(inputs["rp_w"], dtype=np.float32)
    rp_b = np.asarray(inputs["rp_b"], dtype=np.float32)

    # base layout for the stacked-tap lhsT (see module docstring):
    #   cols 0:192  : [p = cin + 64*dy(0/1), n, dx*64 + cout]
    #   cols 192:288: [p = cin (0..63),      n, dx*64 + cout]  (kernel row 2)
    base = np.zeros((128, NEXP, CWF), np.float32)
    b01 = conv_w[:, :, :, 0:2, :].transpose(3, 2, 0, 4, 1)  # dy,c,n,dx,m
    base[:, :, 0:192] = b01.reshape(128, NEXP, 192)
    b2 = conv_w[:, :, :, 2, :].transpose(2, 0, 3, 1)        # c,n,dx,m
    base[0:64, :, 192:384] = b2.reshape(64, NEXP, 192)

    blob = np.zeros((128, CBLOB), np.float32)
    blob[:, C_RPW:C_RPW + 512] = (
        rp_w.T.reshape(4, 128, D).transpose(1, 0, 2).reshape(128, 512))
    blob[:, C_RPB] = rp_b
    blob[:, C_ID:C_ID + 128] = np.eye(128, dtype=np.float32)
    blob[0:NEXP, C_EMB:C_EMB + 128] = emb
    selp = np.zeros((B_LOC, NPAIR, 128), np.float32)
    for p in range(NPAIR):
        selp[2 * p, p, 0:64] = 1.0
        selp[2 * p + 1, p, 64:128] = 1.0
    blob[0:B_LOC, C_SELP:C_SELP + 256] = selp.reshape(B_LOC, 256)
    seli = np.zeros((B_LOC, B_LOC, 128), np.float32)
    for i in range(B_LOC):
        seli[i, i, :] = 1.0
    blob[0:B_LOC, C_SELI:C_SELI + 512] = seli.reshape(B_LOC, 512)
    blob[:, C_CBT:C_CBT + NEXP] = np.tile(conv_b.T, (2, 1))

    in_maps = []
    for c in range(N_CORES):
        sl = slice(B_LOC * c, B_LOC * (c + 1))
        cblob = blob.copy()
        cblob[:, C_RV:C_RV + 16] = (
            rv[sl].T.reshape(4, 128, B_LOC).transpose(1, 0, 2).reshape(128, 16))
        in_maps.append({
            "x": np.concatenate([x[sl].reshape(-1),
                                 np.zeros(XPAD, np.float32)]),
            "cst": cblob,
            "base": base,
        })
    return in_maps


def run(inputs, trace=False, **kw):
    """Returns (full_output, BassKernelResults)."""
    nc = _nc_cached()
    in_maps = _prep_in_maps(inputs)
    res = run_bass_kernel_spmd(nc, in_maps, core_ids=list(range(N_CORES)),
                               trace=trace, **kw)
    outs = [r["out"].reshape(B_LOC, COUT, 64, 64)[:, :, :62, :62]
            for r in res.results]
    return np.concatenate(outs, axis=0), res


def kernel(**inputs):
    out, _ = run(inputs, trace=False)
    return out
